# revision 26
# baseline (speedup 1.0000x reference)
"""Trainium2 Bass kernel for the CSDGI encoder/decoder (2-cluster graph message passing).

Data-parallel over batch: B=512 -> 64 rows per core on 8 NeuronCores.

Per-core dataflow (v2, "flipped" orientation + fp8 DoubleRow main matmul):
  encoder  : 4-way block-diagonal batched dense stack in bf16
             ([128, 512] tiles hold 4 batch rows x 32 channels), residual
             folded into weights (W+I), biases via per-partition ACT bias
  transpose: PE transposes [128,128] -> quad PSUM [128,512] -> single
             scatter-eviction into fp8 XT tile [128, tt, s, b, p]
  graph G  : on-device: row-normalize gw, gram wn^T wn, |.|, row-norm,
             diag zero (affine_select on f32), scale by GS -> fp8e4
  main     : out[n, bp] = sum_m G[m, n] * x[m, bp] as fp8 DoubleRow
             matmuls (K=256 per instruction, 0.5 cycles/row)
  tanh     : ACT Tanh(psum * 1/(10*GS) + gb/10) -> bf16; gb bias is
             per-partition in this orientation (free)
  decoder  : DVE mult by fw (broadcast over b), fold 96->48->24,
             pool_avg over 24 -> y[n, b]; fb + PE transpose at the tail
"""

import numpy as np
import ml_dtypes

import concourse.bass as bass
from concourse import bacc
import concourse.mybir as mybir
from concourse.tile import TileContext

BF16 = mybir.dt.bfloat16
F32 = mybir.dt.float32
FP8 = mybir.dt.float8e4

B, M, P, CH = 512, 1024, 96, 32
NCORES = 8
BL = B // NCORES          # 64 batch rows per core
CB = 16                   # batch rows per chunk
NCHUNKS = BL // CB        # 4 chunks
NG = 4                    # groups of 4 batch rows (block-diag batching)
GS = 512.0                # G fp8 pre-scale
SCALE = 1.0 / (10.0 * GS)  # tanh scale on main psum
EPS = 1e-10
DR = mybir.MatmulPerfMode.DoubleRow

# engine rotation knobs (tuned against TimelineSim)
RELU_ENG = ["act", "vec"]              # l<2 relu evictions (psum: act/vec only)
QUAD_ENG = ["vec", "act"]              # quad evictions (psum: act/vec only)
QUAD_DMA = False                       # PSUM source not DMA-able
FOLD2_ENG = "gps"                      # second fold engine
POOL_ENG = "vec"                       # final window pool engine


def build_nc(n_chunks=NCHUNKS):
    nc = bacc.Bacc(None, target_bir_lowering=False)

    # ---- DRAM I/O ----
    x_d = nc.dram_tensor("x", [NCHUNKS, NG, NG * M], BF16, kind="ExternalInput")
    a0_d = nc.dram_tensor("a0blk", [NG, 128], BF16, kind="ExternalInput")
    w1_d = nc.dram_tensor("w1blk", [128, 128], BF16, kind="ExternalInput")
    w2_d = nc.dram_tensor("w2blk", [128, 128], BF16, kind="ExternalInput")
    encb_d = nc.dram_tensor("encb", [3, 128, 1], F32, kind="ExternalInput")
    gw_d = [nc.dram_tensor(f"gw{c}", [64, M], F32, kind="ExternalInput") for c in range(2)]
    gbb_d = nc.dram_tensor("gbb", [128, 16], F32, kind="ExternalInput")
    fbb_d = nc.dram_tensor("fbb", [128, 16], F32, kind="ExternalInput")
    fwb_d = nc.dram_tensor("fwb", [128, 16 * 96], BF16, kind="ExternalInput")
    identB_d = nc.dram_tensor("identB", [128, 128], BF16, kind="ExternalInput")
    identT_d = nc.dram_tensor("identT", [128, 128], F32, kind="ExternalInput")
    out_d = [nc.dram_tensor(f"out{c}", [BL, M], F32, kind="ExternalOutput") for c in range(2)]

    with TileContext(nc) as tc:
        with (
            tc.tile_pool(name="const", bufs=1) as cst,
            tc.tile_pool(name="setup", bufs=3) as stp,
            tc.tile_pool(name="work", bufs=2) as wrk,
            tc.tile_pool(name="tt", bufs=3) as ttp,
            tc.tile_pool(name="psE", bufs=2, space="PSUM") as psE,
            tc.tile_pool(name="psT", bufs=2, space="PSUM") as psT,
            tc.tile_pool(name="psM", bufs=3, space="PSUM") as psM,
        ):
            # ---- constants into SBUF ----
            gww_s = [stp.tile([64, M], F32, tag=f"gww{c}", name=f"gww{c}")
                     for c in range(2)]
            for c in range(2):
                nc.gpsimd.dma_start(gww_s[c], gw_d[c][:, :])
            a0_s = cst.tile([NG, 128], BF16, tag="a0", name="a0")
            nc.sync.dma_start(a0_s, a0_d[:, :])
            w1_s = cst.tile([128, 128], BF16, tag="w1", name="w1")
            nc.sync.dma_start(w1_s, w1_d[:, :])
            w2_s = cst.tile([128, 128], BF16, tag="w2", name="w2")
            nc.sync.dma_start(w2_s, w2_d[:, :])
            encb_s = []
            for l in range(3):
                t = cst.tile([128, 1], F32, tag=f"encb{l}", name=f"encb{l}")
                nc.sync.dma_start(t, encb_d[l, :, :])
                encb_s.append(t)
            gbb_s = cst.tile([128, 16], F32, tag="gbb", name="gbb")
            nc.sync.dma_start(gbb_s, gbb_d[:, :])
            fbb_s = cst.tile([128, 16], F32, tag="fbb", name="fbb")
            nc.sync.dma_start(fbb_s, fbb_d[:, :])
            fwb_s = cst.tile([128, 16 * 96], BF16, tag="fwb", name="fwb")
            nc.sync.dma_start(fwb_s, fwb_d[:, :])
            identB_s = cst.tile([128, 128], BF16, tag="identB", name="identB")
            nc.sync.dma_start(identB_s, identB_d[:, :])
            identT_s = cst.tile([128, 128], F32, tag="identT", name="identT")
            nc.sync.dma_start(identT_s, identT_d[:, :])

            # persistent: G (fp8, DoubleRow layout) and y accumulators
            G_s = [[cst.tile([128, 2, M], FP8, tag=f"G{c}_{tt}", name=f"G{c}_{tt}")
                    for tt in range(4)] for c in range(2)]
            y_s = [[cst.tile([128, BL], F32, tag=f"y{c}_{nb}", name=f"y{c}_{nb}")
                    for nb in range(8)] for c in range(2)]
            wnb_s = [cst.tile([64, M], BF16, tag=f"wnb{c}", name=f"wnb{c}")
                     for c in range(2)]

            def setup_norm(c):
                def emit():
                    gww = gww_s[c]
                    sq = stp.tile([64, M], F32, tag="sq", name="sq")
                    ss = stp.tile([64, 1], F32, tag="ss", name="ss")
                    nc.scalar.activation(sq, gww,
                                         mybir.ActivationFunctionType.Square,
                                         accum_out=ss)
                    nrm = stp.tile([64, 1], F32, tag="nrm", name="nrm")
                    nc.scalar.activation(nrm, ss,
                                         mybir.ActivationFunctionType.Sqrt)
                    nc.vector.tensor_scalar_max(nrm, nrm, EPS)
                    rinv = stp.tile([64, 1], F32, tag="rinv", name="rinv")
                    nc.vector.reciprocal(rinv, nrm)
                    nc.scalar.activation(wnb_s[c], gww,
                                         mybir.ActivationFunctionType.Copy,
                                         scale=rinv)
                return emit

            def setup_gmt(c, mt):
                def emit():
                    wnb = wnb_s[c]
                    gf = stp.tile([128, M], F32, tag="gf", name="gf")
                    rs = [stp.tile([128, 1], F32, tag=f"rs{h}", name=f"rs{h}")
                          for h in range(2)]
                    for h in range(2):
                        pg = psM.tile([128, 512], F32, tag="pm", name="pm")
                        nc.tensor.matmul(pg, lhsT=wnb[:, mt * 128:(mt + 1) * 128],
                                         rhs=wnb[:, h * 512:(h + 1) * 512],
                                         start=True, stop=True)
                        nc.scalar.activation(
                            gf[:, h * 512:(h + 1) * 512], pg,
                            mybir.ActivationFunctionType.Abs,
                            accum_out=rs[h])
                    rsum = stp.tile([128, 1], F32, tag="rsum", name="rsum")
                    nc.vector.tensor_add(rsum, rs[0], rs[1])
                    rsinv = stp.tile([128, 1], F32, tag="rsinv", name="rsinv")
                    nc.vector.reciprocal(rsinv, rsum)
                    rsinv2 = stp.tile([128, 1], F32, tag="rsinv2", name="rsinv2")
                    nc.vector.tensor_scalar_mul(rsinv2, rsinv, GS)
                    # zero diagonal on f32 rows, then scale-convert to fp8
                    nc.gpsimd.affine_select(
                        out=gf, in_=gf,
                        compare_op=mybir.AluOpType.not_equal, fill=0.0,
                        base=mt * 128, channel_multiplier=1, pattern=[[-1, M]])
                    nc.vector.tensor_scalar(G_s[c][mt // 2][:, mt % 2, :], gf,
                                            rsinv2, None,
                                            op0=mybir.AluOpType.mult)
                return emit

            setup_units0 = [setup_norm(0)] + [setup_gmt(0, mt) for mt in range(8)]
            setup_units1 = [setup_norm(1)] + [setup_gmt(1, mt) for mt in range(8)]

            # ---- helpers for engine rotation ----
            def evict_relu(eng, dst, src, bias):
                if eng == "act":
                    nc.scalar.activation(dst, src,
                                         mybir.ActivationFunctionType.Relu,
                                         bias=bias)
                elif eng == "vec":
                    nc.vector.tensor_scalar(dst, src, bias, 0.0,
                                            op0=mybir.AluOpType.add,
                                            op1=mybir.AluOpType.max)
                else:
                    nc.gpsimd.tensor_scalar(dst, src, bias, 0.0,
                                            op0=mybir.AluOpType.add,
                                            op1=mybir.AluOpType.max)

            def evict_copy(eng, dst, src):
                if eng == "act":
                    nc.scalar.activation(dst, src,
                                         mybir.ActivationFunctionType.Copy)
                elif eng == "vec":
                    nc.vector.tensor_copy(dst, src)
                else:
                    nc.gpsimd.tensor_copy(dst, src)

            # ---- encoder + transpose steps for one chunk ----
            def encoder_steps(cb):
                """Returns (XT, steps). XT is [128, 4(tt), 2(s), CB, 96] fp8."""
                relu_rot, relu2_rot, quad_rot = \
                    RELU_ENG, ["vec", "act"], QUAD_ENG
                xg = wrk.tile([NG, NG * M], BF16, tag="xg", name="xg", bufs=2)
                XT = wrk.tile([128, 4, 2, CB, 96], FP8, tag="XT", name="XT",
                              bufs=2)
                E = {}
                steps = [lambda: nc.sync.dma_start(xg, x_d[cb])]
                ei = [0]

                def enc_step(l, g, t):
                    def emit():
                        pE = psE.tile([128, 512], F32, tag="pE", name="pE")
                        if l == 0:
                            rhs = xg[:, g * M + t * 512: g * M + t * 512 + 512]
                            nc.tensor.matmul(pE, lhsT=a0_s, rhs=rhs,
                                             start=True, stop=True,
                                             tile_position=(0, 0))
                        else:
                            w = w1_s if l == 1 else w2_s
                            nc.tensor.matmul(pE, lhsT=w, rhs=E[(l - 1, g, t)],
                                             start=True, stop=True)
                        Et = wrk.tile([128, 512], BF16, tag=f"E{l}_{g}_{t}",
                                      name=f"E{l}_{g}_{t}", bufs=2)
                        E[(l, g, t)] = Et
                        if l == 2:
                            eng = relu2_rot[(g * 2 + t) % len(relu2_rot)]
                        else:
                            eng = relu_rot[ei[0] % len(relu_rot)]
                            ei[0] += 1
                        evict_relu(eng, Et, pE, encb_s[l])
                    return emit

                pt_state = {"tile": None}

                def transp_step(l, g, t, half):
                    def emit():
                        if t == 0 and half == 0:
                            pt_state["tile"] = psT.tile(
                                [128, 1024], BF16, tag="pT", name="pT")
                        slot = t * 512
                        pt = pt_state["tile"]
                        Et = E[(l, g, t)]
                        for q in (2 * half, 2 * half + 1):
                            nc.tensor.transpose(
                                pt[:, slot + q * 128:slot + (q + 1) * 128],
                                Et[:, q * 128:(q + 1) * 128], identB_s)
                        if t == 1 and half == 1:
                            # both t-quads of (l, g): dst covers all 4 tt slots
                            dst = XT[:, :, :, 4 * g:4 * g + 4,
                                     32 * l:32 * l + 32]
                            kq = l * 4 + g
                            evict_copy(quad_rot[kq % len(quad_rot)], dst, pt)
                    return emit

                for g in range(NG):
                    for l in range(3):
                        for t in range(2):
                            steps.append(enc_step(l, g, t))
                            steps.append(transp_step(l, g, t, 0))
                            steps.append(transp_step(l, g, t, 1))
                return XT, steps

            # ---- main compute for one chunk ----
            tail_done = set()

            def emit_tail(c, nb):
                k = c * 8 + nb
                yt = ttp.tile([128, BL], F32, tag="yt", name="yt")
                nc.gpsimd.tensor_scalar_add(yt, y_s[c][nb],
                                            fbb_s[:, k:k + 1])
                py = psE.tile([BL, 128], F32, tag="py", name="py", bufs=1)
                nc.tensor.transpose(py, yt, identT_s)
                evict_copy("act" if nb % 2 == 0 else "vec",
                           osb[c][:, nb * 128:(nb + 1) * 128], py)
                tail_done.add((c, nb))

            def emit_main(cb, XT, fillers, last=False):
                fillers = list(fillers)
                # 16 groups x 3 fill slots; spread fillers evenly over them
                nslots = 48.0
                quota = [0.0, len(fillers) / nslots]

                def fill(k=None):
                    if k is not None:
                        for _ in range(k):
                            if fillers:
                                fillers.pop(0)()
                        return
                    quota[0] += quota[1]
                    while quota[0] >= 1.0 and fillers:
                        quota[0] -= 1.0
                        fillers.pop(0)()

                for c in range(2):
                    for nb in range(8):
                        k = c * 8 + nb
                        T = ttp.tile([128, 3 * 512], BF16, tag="T", name="T")
                        for h in range(3):
                            pm = psM.tile([128, 512], F32, tag="pm", name="pm")
                            for tt in range(4):
                                rhs = XT[:, tt].rearrange(
                                    "p s b q -> p s (b q)")[:, :,
                                                            512 * h:512 * (h + 1)]
                                nc.tensor.matmul(
                                    pm, lhsT=G_s[c][tt][:, :,
                                                        nb * 128:(nb + 1) * 128],
                                    rhs=rhs, start=(tt == 0), stop=(tt == 3),
                                    perf_mode=DR)
                                if tt == 1:
                                    fill()
                            fill()
                            nc.scalar.activation(
                                T[:, h * 512:(h + 1) * 512], pm,
                                mybir.ActivationFunctionType.Tanh,
                                bias=gbb_s[:, k:k + 1], scale=SCALE)
                        Tv = T.rearrange("p (b q) -> p b q", b=CB)
                        fw = fwb_s[:, k * 96:(k + 1) * 96].unsqueeze(1) \
                            .broadcast_to((128, CB, 96))
                        T2 = ttp.tile([128, CB, 96], BF16, tag="T2", name="T2")
                        nc.vector.tensor_mul(T2, Tv, fw)
                        T3 = ttp.tile([128, CB, 64], BF16, tag="T3", name="T3")
                        nc.gpsimd.tensor_add(T3[:, :, 0:48], T2[:, :, 0:48],
                                             T2[:, :, 48:96])
                        ypool = y_s[c][nb][:, cb * CB:(cb + 1) * CB]
                        nc.vector.tensor_reduce(ypool, T3[:, :, 0:48],
                                                mybir.AxisListType.X,
                                                mybir.AluOpType.add)
                        if last:
                            emit_tail(c, nb)
                        fill()
                fill(len(fillers))

            osb = [wrk.tile([BL, M], F32, tag=f"osb{c}", name=f"osb{c}", bufs=1)
                   for c in range(2)]

            # ---- schedule: prologue = chunk 0 encoder + setup interleave ----
            XT_of = {}
            XT_of[0], s0 = encoder_steps(0)
            su = list(setup_units0)
            su.pop(0)()
            setup_units1[0]()   # norm(1): keep all Sqrt before first Tanh
            for kk, f in enumerate(s0):
                f()
                if su:
                    su.pop(0)()
            for f in su:
                f()
            for cb in range(n_chunks):
                if cb + 1 < n_chunks:
                    XT_of[cb + 1], fillers = encoder_steps(cb + 1)
                else:
                    fillers = []
                if cb == 0:
                    fillers = setup_units1[1:] + fillers
                emit_main(cb, XT_of.pop(cb), fillers,
                          last=(cb == n_chunks - 1))

            # ---- tail: any groups not already emitted inline, then store ----
            for c in range(2):
                for nb in range(8):
                    if (c, nb) not in tail_done:
                        emit_tail(c, nb)
                nc.sync.dma_start(out_d[c][:, :], osb[c])

    nc.compile()
    return nc


def _bf(x):
    return np.asarray(x, dtype=np.float32).astype(ml_dtypes.bfloat16)


F8NP = mybir.dt.np(FP8)


def _f8(x):
    return np.asarray(x, dtype=np.float32).astype(F8NP)


def prep_params(inputs):
    """Host-side layout/dtype prep of the small replicated parameters."""
    w0 = np.asarray(inputs["w0"], dtype=np.float32)   # [32, 1]
    w1 = np.asarray(inputs["w1"], dtype=np.float32)   # [32, 32]
    w2 = np.asarray(inputs["w2"], dtype=np.float32)
    I = np.eye(CH, dtype=np.float32)
    I4 = np.eye(NG, dtype=np.float32)
    a0 = (w0[:, 0] + 1.0).reshape(1, CH)
    p = {
        "a0blk": _bf(np.kron(I4, a0)),                     # [4, 128]
        "w1blk": _bf(np.kron(I4, (w1 + I).T)),             # [128, 128]
        "w2blk": _bf(np.kron(I4, (w2 + I).T)),
    }
    encb = np.zeros((3, 128, 1), dtype=np.float32)
    for l, bk in enumerate(("b0", "b1", "b2")):
        encb[l, :, 0] = np.tile(np.asarray(inputs[bk], dtype=np.float32), NG)
    p["encb"] = encb
    gbb = np.zeros((128, 16), dtype=np.float32)
    fbb = np.zeros((128, 16), dtype=np.float32)
    fwb = np.zeros((128, 16 * 96), dtype=ml_dtypes.bfloat16)
    for c in range(2):
        gb = np.asarray(inputs[f"gb{c}"], dtype=np.float32)
        fb = np.asarray(inputs[f"fb{c}"], dtype=np.float32)[:, 0]
        fw = np.asarray(inputs[f"fw{c}"], dtype=np.float32)[:, :, 0]  # [M, P]
        p[f"gw{c}"] = np.asarray(inputs[f"gw{c}"], dtype=np.float32)
        for nb in range(8):
            k = c * 8 + nb
            sl = slice(nb * 128, (nb + 1) * 128)
            gbb[:, k] = gb[sl] * 0.1
            fbb[:, k] = fb[sl]
            fwb[:, k * 96:(k + 1) * 96] = _bf(fw[sl, :])
    p["gbb"] = gbb
    p["fbb"] = fbb
    p["fwb"] = fwb
    p["identB"] = np.eye(128, dtype=ml_dtypes.bfloat16)
    p["identT"] = np.eye(128, dtype=np.float32)
    return p


def prep_x(x_core):
    """[BL, M] f32 -> [NCHUNKS, NG, NG*M] fp8 in [cb][p][g][m] order."""
    xr = _bf(x_core).reshape(NCHUNKS, NG, NG, M)      # [cb][g][p][m]
    return np.ascontiguousarray(xr.transpose(0, 2, 1, 3))  # [cb][p][g][m]


def build_in_maps(inputs):
    params = prep_params(inputs)
    x = np.asarray(inputs["inputs"], dtype=np.float32)[:, :, 0]   # [B, M]
    in_maps = []
    for i in range(NCORES):
        m = dict(params)
        m["x"] = prep_x(x[i * BL:(i + 1) * BL])
        in_maps.append(m)
    return in_maps


_NC_CACHE = {}


def run(inputs, **kw):
    from concourse.bass_utils import run_bass_kernel_spmd

    if "nc" not in _NC_CACHE:
        _NC_CACHE["nc"] = build_nc()
    nc = _NC_CACHE["nc"]

    in_maps = build_in_maps(inputs)
    res = run_bass_kernel_spmd(nc, in_maps, core_ids=list(range(NCORES)), **kw)
    y0 = np.concatenate([res.results[i]["out0"] for i in range(NCORES)], axis=0)
    y1 = np.concatenate([res.results[i]["out1"] for i in range(NCORES)], axis=0)
    return (y0.astype(np.float32), y1.astype(np.float32)), res


def kernel(**inputs):
    outs, _ = run(inputs)
    return outs
